# revision 1
# baseline (speedup 1.0000x reference)
"""MoE (top-2 of 8 experts) SwiGLU FFN on 8 Trainium2 NeuronCores.

Strategy (expert-parallel, per the sharding hint):
  - Router (x @ w_gate -> softmax -> top-2) computed host-side on jax-CPU with
    the exact ops the reference uses, so expert selection matches the
    reference bit-for-bit. This is the "dispatch tokens by topk_idx" step.
  - Core e receives only the tokens routed to expert e (gathered, transposed,
    and pre-cast to bf16 host-side), plus expert e's weights pre-packed into
    the SBUF tile layout (so every device DMA is a single contiguous 2D
    HWDGE transfer). All cores run one SPMD program sized to
    cap = max tokens per expert (zero-padded).
  - Device computes y_e^T = wo_e^T @ (silu(wg_e^T x^T) * (wi_e^T x^T)) with
    bf16 matmuls accumulating in fp32 PSUM. Tokens stay on the PSUM free
    dimension throughout, so no on-device transposes are needed: lhsT
    operands are the natural wi/wg [C,H] and wo [H,C] layouts.
  - Host combines: out[t] = val0[t]*y_{e0}[t] + val1[t]*y_{e1}[t].
"""

import numpy as np
import ml_dtypes

import concourse.bass as bass
import concourse.mybir as mybir
import concourse.tile as tile
from concourse.bass_utils import run_bass_kernel_spmd

N_CORES = 8
N_EXPERTS = 8
TOP_K = 2
B, T, C, H = 4, 2048, 1024, 2048
CC = C // 128           # contraction chunks over C
HH = H // 128           # chunks over H
TOK_TILE = 512          # tokens per PSUM tile (one fp32 bank)
HBW = 512               # stage-1 weight block width (columns of H)
CBW = 512               # stage-2 weight block width (columns of C)
HB = H // HBW
CB = C // CBW
BF16 = mybir.dt.bfloat16


def _split_multi_waits(nc, max_waits=1):
    """This walrus build rejects >1 sync-wait per instruction. Peel extra
    waits onto single-wait EventSemaphore instructions inserted just before,
    on the same engine (identical blocking semantics)."""
    n_split = 0
    for fn in nc.m.functions:
        for bb in fn.blocks:
            out = []
            changed = False
            for inst in bb.instructions:
                si = inst.sync_info
                waits = list(si.on_wait) if si is not None else []
                if len(waits) > max_waits:
                    head, keep = waits[:-max_waits], waits[-max_waits:]
                    for j, w in enumerate(head):
                        out.append(mybir.InstEventSemaphore(
                            name=f"{inst.name}-wspl{j}",
                            engine=inst.engine,
                            sync_info=mybir.SyncInfo(on_wait=[w], on_update=[]),
                        ))
                    inst.sync_info = mybir.SyncInfo(
                        on_wait=keep, on_update=list(si.on_update))
                    changed = True
                    n_split += 1
                out.append(inst)
            if changed:
                bb.instructions = out
    return n_split


def build_program(cap, reps=1):
    """One SPMD program: expert FFN over [cap] tokens (token dim = PSUM free
    dim everywhere). reps>1 repeats the whole compute (timing only).

    DRAM inputs are already in SBUF tile layout, bf16:
      xtb [128, CC*cap]     token-tile-major: for each 512-token tile tt,
                            a [128, CC*tw] slab with xtb[p, cc*tw+t] =
                            x^T[cc*128+p, tt*512+t] — so stage 1 can start
                            after the first ~1MB tile lands instead of the
                            full slab
      wib [128, HB*CC*HBW]  wib[p, (hb*CC+cc)*HBW+f] = wi[cc*128+p, hb*HBW+f]
      wgb [128, HB*CC*HBW]  same layout as wib
      wob [128, CB*HH*CBW]  wob[p, (cb*HH+hh)*CBW+f] = wo[hh*128+p, cb*CBW+f]
    Output yt [C, cap] fp32 (y^T, one row block per c-chunk).
    """
    assert cap % 4 == 0
    nc = bass.Bass()
    xtb = nc.dram_tensor("xtb", [128, CC * cap], BF16, kind="ExternalInput")
    wib = nc.dram_tensor("wib", [128, HB * CC * HBW], BF16, kind="ExternalInput")
    wgb = nc.dram_tensor("wgb", [128, HB * CC * HBW], BF16, kind="ExternalInput")
    wob = nc.dram_tensor("wob", [128, CB * HH * CBW], BF16, kind="ExternalInput")
    yt = nc.dram_tensor("yt", [C, cap], mybir.dt.float32, kind="ExternalOutput")
    # tiny output: fetching it waits for program completion without paying
    # the 9 MB/core yt transfer through the tunnel (timing use)
    done = nc.dram_tensor("done", [1, 8], mybir.dt.float32,
                          kind="ExternalOutput")

    tok_tiles = [(t0, min(TOK_TILE, cap - t0)) for t0 in range(0, cap, TOK_TILE)]
    BLK = CC * HBW          # elements per (hb, cc-full) stage-1 block
    BLK2 = HH * CBW         # elements per (cb, hh-full) stage-2 block

    with tile.TileContext(nc) as tc:
        with tc.tile_pool(name="xb", bufs=1) as xb_pool, \
             tc.tile_pool(name="w1", bufs=2) as w1_pool, \
             tc.tile_pool(name="hT", bufs=1) as h_pool, \
             tc.tile_pool(name="w2", bufs=2) as w2_pool, \
             tc.tile_pool(name="sg", bufs=3) as sg_pool, \
             tc.tile_pool(name="yo", bufs=3) as yo_pool, \
             tc.tile_pool(name="ps", bufs=2, space="PSUM") as ps_pool, \
             tc.tile_pool(name="ps2", bufs=3, space="PSUM") as ps2_pool:

            for _rep in range(reps):
                # one SBUF tile per token tile, loaded in order so the first
                # matmuls wait only on the first ~1MB slab
                xts = []
                off = 0
                for t0, tw in tok_tiles:
                    xt_t = xb_pool.tile([128, CC * tw], BF16,
                                        tag=f"xb{t0 // TOK_TILE}")
                    nc.sync.dma_start(xt_t[:], xtb[:, off:off + CC * tw])
                    xts.append(xt_t)
                    off += CC * tw

                # hT = silu(x@wg) * (x@wi), transposed: [H, cap] bf16
                hT = h_pool.tile([128, HH * cap], BF16, tag="hT")

                # ---- stage 1 ----
                for hb in range(HB):
                    wib_t = w1_pool.tile([128, BLK], BF16, tag="wib")
                    nc.sync.dma_start(wib_t[:],
                                      wib[:, hb * BLK:(hb + 1) * BLK])
                    wgb_t = w1_pool.tile([128, BLK], BF16, tag="wgb")
                    nc.sync.dma_start(wgb_t[:],
                                      wgb[:, hb * BLK:(hb + 1) * BLK])
                    for hi in range(HBW // 128):
                        hh = hb * (HBW // 128) + hi
                        for ti, (t0, tw) in enumerate(tok_tiles):
                            ps_u = ps_pool.tile([128, TOK_TILE],
                                                mybir.dt.float32, tag="psu")
                            ps_g = ps_pool.tile([128, TOK_TILE],
                                                mybir.dt.float32, tag="psg")
                            for cc in range(CC):
                                nc.tensor.matmul(
                                    ps_u[:, :tw],
                                    wib_t[:, cc * HBW + hi * 128:
                                          cc * HBW + (hi + 1) * 128],
                                    xts[ti][:, cc * tw:(cc + 1) * tw],
                                    start=(cc == 0), stop=(cc == CC - 1))
                            for cc in range(CC):
                                nc.tensor.matmul(
                                    ps_g[:, :tw],
                                    wgb_t[:, cc * HBW + hi * 128:
                                          cc * HBW + (hi + 1) * 128],
                                    xts[ti][:, cc * tw:(cc + 1) * tw],
                                    start=(cc == 0), stop=(cc == CC - 1))
                            sg = sg_pool.tile([128, TOK_TILE],
                                              mybir.dt.float32, tag="sg")
                            nc.scalar.activation(
                                sg[:, :tw], ps_g[:, :tw],
                                mybir.ActivationFunctionType.Silu)
                            nc.vector.tensor_mul(
                                hT[:, hh * cap + t0: hh * cap + t0 + tw],
                                ps_u[:, :tw], sg[:, :tw])

                # ---- stage 2: yT = wo^T @ hT ----
                for cb in range(CB):
                    wob_t = w2_pool.tile([128, BLK2], BF16, tag="wob")
                    nc.sync.dma_start(wob_t[:],
                                      wob[:, cb * BLK2:(cb + 1) * BLK2])
                    for ci in range(CBW // 128):
                        c0 = cb * CBW + ci * 128
                        for t0, tw in tok_tiles:
                            ps_y = ps2_pool.tile([128, TOK_TILE],
                                                 mybir.dt.float32, tag="psy")
                            for hh in range(HH):
                                nc.tensor.matmul(
                                    ps_y[:, :tw],
                                    wob_t[:, hh * CBW + ci * 128:
                                          hh * CBW + (ci + 1) * 128],
                                    hT[:, hh * cap + t0: hh * cap + t0 + tw],
                                    start=(hh == 0), stop=(hh == HH - 1))
                            yo = yo_pool.tile([128, TOK_TILE],
                                              mybir.dt.float32, tag="yo")
                            nc.vector.tensor_copy(yo[:, :tw], ps_y[:, :tw])
                            nc.sync.dma_start(yt[c0:c0 + 128, t0:t0 + tw],
                                              yo[:, :tw])
                            if cb == CB - 1 and ci == CBW // 128 - 1 \
                                    and t0 + tw == cap:
                                nc.sync.dma_start(done[0:1, 0:8], yo[0:1, 0:8])
    _split_multi_waits(nc)
    return nc


def pack_wi(w):
    """wi/wg [C, H] f32 -> [128, HB*CC*HBW] bf16 in the wib DRAM layout."""
    a = np.asarray(w).reshape(CC, 128, HB, HBW)          # [cc, p, hb, f]
    a = a.transpose(1, 2, 0, 3)                          # [p, hb, cc, f]
    return np.ascontiguousarray(a.reshape(128, HB * CC * HBW)
                                ).astype(ml_dtypes.bfloat16)


def pack_wo(w):
    """wo [H, C] f32 -> [128, CB*HH*CBW] bf16 in the wob DRAM layout."""
    a = np.asarray(w).reshape(HH, 128, CB, CBW)          # [hh, p, cb, f]
    a = a.transpose(1, 2, 0, 3)                          # [p, cb, hh, f]
    return np.ascontiguousarray(a.reshape(128, CB * HH * CBW)
                                ).astype(ml_dtypes.bfloat16)


def pack_x(x_disp_T):
    """x^T dispatch slab [C, cap] f32 -> [128, CC*cap] bf16 in the
    token-tile-major xtb layout (see build_program docstring)."""
    cap = x_disp_T.shape[1]
    a = x_disp_T.reshape(CC, 128, cap)                      # [cc, p, t]
    parts = []
    for t0 in range(0, cap, TOK_TILE):
        tw = min(TOK_TILE, cap - t0)
        blk = a[:, :, t0:t0 + tw].transpose(1, 0, 2)        # [p, cc, tw]
        parts.append(blk.reshape(128, CC * tw))
    return np.ascontiguousarray(np.concatenate(parts, axis=1)
                                ).astype(ml_dtypes.bfloat16)


def _route(x, w_gate):
    """Host-side router. Runs the exact reference ops on jax-CPU so the
    top-2 selection and gate values match the reference bit-for-bit."""
    import jax
    import jax.numpy as jnp
    cpu = jax.devices("cpu")[0]
    with jax.default_device(cpu):
        xj = jnp.asarray(np.asarray(x))
        wj = jnp.asarray(np.asarray(w_gate))
        logits = jnp.einsum("btc,ce->bte", xj, wj)
        gates = jax.nn.softmax(logits, axis=-1)
        topk_vals, topk_idx = jax.lax.top_k(gates, TOP_K)
    return (np.asarray(topk_vals).reshape(-1, TOP_K),
            np.asarray(topk_idx).reshape(-1, TOP_K))


def _dispatch(x, topk_idx):
    """Token lists per expert, (token, slot) positions, cap, and the
    gathered+packed per-expert xtb slabs."""
    N = x.shape[0] * x.shape[1] if x.ndim == 3 else x.shape[0]
    x_flat = np.ascontiguousarray(np.asarray(x).reshape(N, C))
    idx_lists = []
    pos = np.empty((N, TOP_K), dtype=np.int64)
    for e in range(N_EXPERTS):
        sel = (topk_idx == e)
        toks = np.flatnonzero(sel.any(axis=1))
        idx_lists.append(toks)
        pos_of = np.full(N, -1, dtype=np.int64)
        pos_of[toks] = np.arange(len(toks))
        for k in range(TOP_K):
            m = sel[:, k]
            pos[m, k] = pos_of[m]
    max_cnt = max(len(t) for t in idx_lists)
    cap = max(128, -(-max_cnt // 4) * 4)

    xT = np.ascontiguousarray(x_flat.T)            # [C, N]
    xtbs = []
    for e in range(N_EXPERTS):
        toks = idx_lists[e]
        slab = np.zeros((C, cap), dtype=np.float32)
        slab[:, :len(toks)] = xT[:, toks]
        xtbs.append(pack_x(slab))
    return idx_lists, pos, cap, xtbs


def make_in_maps(x, wi, wg, wo, topk_idx):
    idx_lists, pos, cap, xtbs = _dispatch(x, topk_idx)
    in_maps = []
    for e in range(N_EXPERTS):
        in_maps.append({
            "xtb": xtbs[e],
            "wib": pack_wi(wi[e]),
            "wgb": pack_wi(wg[e]),
            "wob": pack_wo(wo[e]),
        })
    return idx_lists, pos, cap, in_maps


def kernel(x, w_gate, wi, wg, wo):
    x = np.asarray(x)
    wi, wg, wo = np.asarray(wi), np.asarray(wg), np.asarray(wo)
    N = B * T

    topk_vals, topk_idx = _route(x, w_gate)
    idx_lists, pos, cap, in_maps = make_in_maps(x, wi, wg, wo, topk_idx)

    nc = build_program(cap)
    res = run_bass_kernel_spmd(nc, in_maps, core_ids=list(range(N_CORES)))

    # combine: out[t] = sum_k vals[t,k] * y_{idx[t,k]}[t]
    Y = np.empty((N_EXPERTS, cap, C), dtype=np.float32)   # token-major
    for e in range(N_EXPERTS):
        Y[e] = res.results[e]["yt"].T
    out = (topk_vals[:, 0:1] * Y[topk_idx[:, 0], pos[:, 0], :]
           + topk_vals[:, 1:2] * Y[topk_idx[:, 1], pos[:, 1], :])
    return out.reshape(B, T, C).astype(np.float32)



# revision 10
# speedup vs baseline: 2.8268x; 2.8268x over previous
"""MoE (top-2 of 8 experts) SwiGLU FFN on 8 Trainium2 NeuronCores.

Strategy — expert-parallel with H-sharding for perfect load balance:
  - Router (x @ w_gate -> softmax -> top-2) computed host-side on jax-CPU
    with the exact ops the reference uses, so expert selection matches the
    reference bit-for-bit ("dispatch tokens by topk_idx").
  - Every core processes ALL 8 experts; core c owns H-chunk c (256 of 2048
    H columns) of every expert's weights.  Slot j of the SPMD program is
    expert j with exactly cap_j = round4(load_j) tokens, so per-core PE
    cycles sit at the arithmetic floor (sum of loads / 8 cores) instead of
    8 x max-expert-load: measured ~5% faster than per-expert-per-core
    sharding, whose cap is the max expert load.
  - Device computes, per slot, partial y^T = wo_c^T @ (silu(wg_c^T x^T) *
    (wi_c^T x^T)) with bf16 matmuls accumulating in fp32 PSUM.  Tokens stay
    on the PSUM free dimension throughout so no on-device transposes are
    needed.  Partial y (this core's H-chunk contribution) is written bf16;
    the host sums the 8 cores' partials in fp32 and applies the top-2
    combine weights.
  - Perf notes baked in below: chip-level power arbitration throttles the
    PE clock when all 8 cores stream matmuls (2.4 -> ~1.9 GHz), so total
    PE cycles, not schedule, dominate; many small output DMAs saturate the
    issuing queue and stall PSUM drain (hence one coalesced DMA per
    (slot, out-chunk)); warm-up matmuls cover the initial DMA wait and
    pre-warm the HAM clock gate.
"""

import numpy as np
import ml_dtypes

import concourse.bass as bass
import concourse.mybir as mybir
import concourse.tile as tile
from concourse.bass_utils import run_bass_kernel_spmd

N_CORES = 8
N_EXPERTS = 8
TOP_K = 2
B, T, C, H = 4, 2048, 1024, 2048
CC = C // 128            # contraction chunks over C
TOK_TILE = 512           # max tokens per PSUM tile (one fp32 bank)
HSH = H // N_CORES       # 256 H columns per core per expert
HSHH = HSH // 128        # hh chunks per slot
CO = C // 128            # output row chunks
BF16 = mybir.dt.bfloat16
W1BLK = HSHH * CC * 128  # stage-1 weight cols per slot
W2BLK = CO * HSHH * 128  # stage-2 weight cols per slot


def _split_multi_waits(nc, max_waits=1):
    """This walrus build rejects >1 sync-wait per instruction. Peel extra
    waits onto single-wait EventSemaphore instructions inserted just before,
    on the same engine (identical blocking semantics)."""
    n_split = 0
    for fn in nc.m.functions:
        for bb in fn.blocks:
            out = []
            changed = False
            for inst in bb.instructions:
                si = inst.sync_info
                waits = list(si.on_wait) if si is not None else []
                if len(waits) > max_waits:
                    head, keep = waits[:-max_waits], waits[-max_waits:]
                    for j, w in enumerate(head):
                        out.append(mybir.InstEventSemaphore(
                            name=f"{inst.name}-wspl{j}",
                            engine=inst.engine,
                            sync_info=mybir.SyncInfo(on_wait=[w], on_update=[]),
                        ))
                    inst.sync_info = mybir.SyncInfo(
                        on_wait=keep, on_update=list(si.on_update))
                    changed = True
                    n_split += 1
                out.append(inst)
            if changed:
                bb.instructions = out
    return n_split


def tok_tiling(cap):
    """Token tiles with a small first tile (fast first x DMA), the rest
    split evenly."""
    if cap <= 256:
        return [(0, cap)]
    first = 128
    rest = cap - first
    k = -(-rest // TOK_TILE)
    w = -(-rest // (4 * k)) * 4
    out = [(0, first)]
    t0 = first
    while t0 < cap:
        tw = min(w, cap - t0)
        out.append((t0, tw))
        t0 += tw
    return out


def tiling_plain(cap):
    """Even split into ceil(cap/512) tiles (multiple-of-4 widths): avoids
    tiny tail tiles whose matmul groups are all fixed overhead."""
    k = -(-cap // TOK_TILE)
    w = -(-cap // (4 * k)) * 4
    out = []
    t0 = 0
    while t0 < cap:
        tw = min(w, cap - t0)
        out.append((t0, tw))
        t0 += tw
    return out


def slot_tiling(j, cap):
    return tok_tiling(cap) if j == 0 else tiling_plain(cap)


def build_program_v3(caps, reps=1):
    """SPMD program over NE slots; slot j = expert j, H-chunk = this core.

    DRAM inputs (bf16), slot-concatenated along columns:
      xtb [128, CC*sum(caps)]   slot j at CC*capoff_j, token-tile-major
                                per slot_tiling(j, cap_j); same on all cores
      wib/wgb [128, NE*W1BLK]   slot j chunk: [p,(hh*CC+cc)*128+f] =
                                wi[j][cc*128+p, c*HSH+hh*128+f]  (core c)
      wob [128, NE*W2BLK]       slot j chunk: [p,(co*HSHH+hh)*128+f] =
                                wo[j][c*HSH+hh*128+p, co*128+f]
    Output ytp [C, sum(caps)] bf16: partial y^T (this core's H-chunk
    contribution), slot j at column capoff_j.  reps>1 repeats the whole
    compute (timing use only).
    """
    NE = N_EXPERTS
    scap = sum(caps)
    capoff = [sum(caps[:j]) for j in range(NE)]

    nc = bass.Bass()
    xtb = nc.dram_tensor("xtb", [128, CC * scap], BF16, kind="ExternalInput")
    wib = nc.dram_tensor("wib", [128, NE * W1BLK], BF16, kind="ExternalInput")
    wgb = nc.dram_tensor("wgb", [128, NE * W1BLK], BF16, kind="ExternalInput")
    wob = nc.dram_tensor("wob", [128, NE * W2BLK], BF16, kind="ExternalInput")
    ytp = nc.dram_tensor("ytp", [C, scap], BF16, kind="ExternalOutput")
    # tiny output: fetching it blocks on program completion without paying
    # the big ytp transfer through the tunnel (timing use)
    done = nc.dram_tensor("done", [1, 8], BF16, kind="ExternalOutput")

    with tile.TileContext(nc) as tc:
        with tc.tile_pool(name="wu", bufs=1) as wu_pool, \
             tc.tile_pool(name="xb", bufs=1) as xb_pool, \
             tc.tile_pool(name="w1", bufs=2) as w1_pool, \
             tc.tile_pool(name="hT", bufs=1) as h_pool, \
             tc.tile_pool(name="w2", bufs=2) as w2_pool, \
             tc.tile_pool(name="sg", bufs=3) as sg_pool, \
             tc.tile_pool(name="yo", bufs=1) as yo_pool, \
             tc.tile_pool(name="psw", bufs=1, space="PSUM") as psw_pool, \
             tc.tile_pool(name="ps", bufs=2, space="PSUM") as ps_pool, \
             tc.tile_pool(name="ps2", bufs=3, space="PSUM") as ps2_pool:

            # warm-up: keep PE busy during the first DMAs (also warms the
            # HAM clock gate); runs on a zeroed scratch tile into a PSUM
            # bank nothing reads
            wup = wu_pool.tile([128, TOK_TILE], BF16, tag="wup")
            nc.vector.memset(wup[:], 0)
            ps_w = psw_pool.tile([128, TOK_TILE], mybir.dt.float32, tag="pw")
            for _ in range(6):
                nc.tensor.matmul(ps_w[:], wup[:, 0:128], wup[:],
                                 start=True, stop=True)

            mcap = max(caps)
            for _rep in range(reps):
                for j in range(NE):
                    cap = caps[j]
                    tiles = slot_tiling(j, cap)
                    xoff = CC * capoff[j]

                    # slot 0: per-tile loads so the first matmul group only
                    # waits on a small transfer; others: one big DMA,
                    # 3-deep rotation so the load runs ~2 slots ahead
                    xs = xb_pool.tile([128, CC * mcap], BF16,
                                      tag=f"x{j % 3}", name=f"xs{j}")
                    if j == 0:
                        off = xoff
                        for t0, tw in tiles:
                            nc.gpsimd.dma_start(
                                xs[:, CC * t0:CC * t0 + CC * tw],
                                xtb[:, off:off + CC * tw])
                            off += CC * tw
                    else:
                        nc.gpsimd.dma_start(xs[:, :CC * cap],
                                            xtb[:, xoff:xoff + CC * cap])
                    xts = [xs[:, CC * t0:CC * (t0 + tw)]
                           for t0, tw in tiles]

                    wib_t = w1_pool.tile([128, W1BLK], BF16, tag="wib")
                    nc.sync.dma_start(
                        wib_t[:], wib[:, j * W1BLK:(j + 1) * W1BLK])
                    wgb_t = w1_pool.tile([128, W1BLK], BF16, tag="wgb")
                    nc.sync.dma_start(
                        wgb_t[:], wgb[:, j * W1BLK:(j + 1) * W1BLK])
                    wob_t = w2_pool.tile([128, W2BLK], BF16, tag="wob")
                    nc.sync.dma_start(
                        wob_t[:], wob[:, j * W2BLK:(j + 1) * W2BLK])

                    hT = h_pool.tile([128, HSHH * mcap], BF16,
                                     tag=f"hT{j % 2}", name=f"hT{j}")

                    # ---- stage 1: hT = silu(x@wg) * (x@wi), H-chunk ----
                    for hh in range(HSHH):
                        for ti, (t0, tw) in enumerate(tiles):
                            ps_u = ps_pool.tile([128, TOK_TILE],
                                                mybir.dt.float32, tag="psu")
                            ps_g = ps_pool.tile([128, TOK_TILE],
                                                mybir.dt.float32, tag="psg")
                            for cc in range(CC):
                                nc.tensor.matmul(
                                    ps_u[:, :tw],
                                    wib_t[:, (hh * CC + cc) * 128:
                                          (hh * CC + cc + 1) * 128],
                                    xts[ti][:, cc * tw:(cc + 1) * tw],
                                    start=(cc == 0), stop=(cc == CC - 1))
                            for cc in range(CC):
                                nc.tensor.matmul(
                                    ps_g[:, :tw],
                                    wgb_t[:, (hh * CC + cc) * 128:
                                          (hh * CC + cc + 1) * 128],
                                    xts[ti][:, cc * tw:(cc + 1) * tw],
                                    start=(cc == 0), stop=(cc == CC - 1))
                            sg = sg_pool.tile([128, TOK_TILE],
                                              mybir.dt.float32, tag="sg")
                            nc.scalar.activation(
                                sg[:, :tw], ps_g[:, :tw],
                                mybir.ActivationFunctionType.Silu)
                            nc.vector.tensor_mul(
                                hT[:, hh * mcap + t0: hh * mcap + t0 + tw],
                                ps_u[:, :tw], sg[:, :tw])

                    # ---- stage 2: partial y^T = wo_chunk^T @ hT ----
                    # PSUM tiles drain into an SBUF strip; one coalesced
                    # DMA per (slot, co) — many small DMAs saturate the
                    # issuing queue and stall PSUM reuse
                    for co in range(CO):
                        c0 = co * 128
                        yo = yo_pool.tile([128, mcap], BF16,
                                          tag=f"yo{co % 2}", name=f"yo{co}")
                        for t0, tw in tiles:
                            ps_y = ps2_pool.tile([128, TOK_TILE],
                                                 mybir.dt.float32, tag="psy")
                            for hh in range(HSHH):
                                nc.tensor.matmul(
                                    ps_y[:, :tw],
                                    wob_t[:, (co * HSHH + hh) * 128:
                                          (co * HSHH + hh + 1) * 128],
                                    hT[:, hh * mcap + t0:
                                       hh * mcap + t0 + tw],
                                    start=(hh == 0), stop=(hh == HSHH - 1))
                            nc.vector.tensor_copy(yo[:, t0:t0 + tw],
                                                  ps_y[:, :tw])
                        nc.scalar.dma_start(
                            ytp[c0:c0 + 128, capoff[j]:capoff[j] + cap],
                            yo[:, :cap])
                        if j == NE - 1 and co == CO - 1:
                            nc.scalar.dma_start(done[0:1, 0:8], yo[0:1, 0:8])
    _split_multi_waits(nc)
    return nc


def pack_wi_v3(w_e, core):
    """wi/wg [C, H] f32, H-chunk of `core` -> [128, W1BLK] bf16."""
    sl = np.asarray(w_e)[:, core * HSH:(core + 1) * HSH]   # [C, HSH]
    a = sl.reshape(CC, 128, HSHH, 128)                     # [cc, p, hh, f]
    a = a.transpose(1, 2, 0, 3)                            # [p, hh, cc, f]
    return np.ascontiguousarray(a.reshape(128, W1BLK)
                                ).astype(ml_dtypes.bfloat16)


def pack_wo_v3(w_e, core):
    """wo [H, C] f32, H-chunk of `core` -> [128, W2BLK] bf16."""
    sl = np.asarray(w_e)[core * HSH:(core + 1) * HSH, :]   # [HSH, C]
    a = sl.reshape(HSHH, 128, CO, 128)                     # [hh, p, co, f]
    a = a.transpose(1, 2, 0, 3)                            # [p, co, hh, f]
    return np.ascontiguousarray(a.reshape(128, W2BLK)
                                ).astype(ml_dtypes.bfloat16)


def pack_x_v3(x_disp_T, j, cap):
    """x^T slab [C, cap] f32 -> [128, CC*cap] bf16 per slot_tiling(j, cap)."""
    a = x_disp_T.reshape(CC, 128, cap)
    parts = []
    for t0, tw in slot_tiling(j, cap):
        blk = a[:, :, t0:t0 + tw].transpose(1, 0, 2)
        parts.append(blk.reshape(128, CC * tw))
    return np.ascontiguousarray(np.concatenate(parts, axis=1)
                                ).astype(ml_dtypes.bfloat16)


def _route(x, w_gate):
    """Host-side router. Runs the exact reference ops on jax-CPU so the
    top-2 selection and gate values match the reference bit-for-bit."""
    import jax
    import jax.numpy as jnp
    cpu = jax.devices("cpu")[0]
    with jax.default_device(cpu):
        xj = jnp.asarray(np.asarray(x))
        wj = jnp.asarray(np.asarray(w_gate))
        logits = jnp.einsum("btc,ce->bte", xj, wj)
        gates = jax.nn.softmax(logits, axis=-1)
        topk_vals, topk_idx = jax.lax.top_k(gates, TOP_K)
    return (np.asarray(topk_vals).reshape(-1, TOP_K),
            np.asarray(topk_idx).reshape(-1, TOP_K))


def make_in_maps_v3(x, wi, wg, wo, topk_idx):
    N = x.shape[0] * x.shape[1] if np.asarray(x).ndim == 3 else x.shape[0]
    x_flat = np.ascontiguousarray(np.asarray(x).reshape(N, C))
    idx_lists = []
    pos = np.empty((N, TOP_K), dtype=np.int64)
    for e in range(N_EXPERTS):
        sel = (topk_idx == e)
        toks = np.flatnonzero(sel.any(axis=1))
        idx_lists.append(toks)
        pos_of = np.full(N, -1, dtype=np.int64)
        pos_of[toks] = np.arange(len(toks))
        for k in range(TOP_K):
            m = sel[:, k]
            pos[m, k] = pos_of[m]
    caps = [max(4, -(-len(t) // 4) * 4) for t in idx_lists]

    xT = np.ascontiguousarray(x_flat.T)
    slabs = []
    for j in range(N_EXPERTS):
        toks = idx_lists[j]
        slab = np.zeros((C, caps[j]), dtype=np.float32)
        slab[:, :len(toks)] = xT[:, toks]
        slabs.append(pack_x_v3(slab, j, caps[j]))
    xtb = np.ascontiguousarray(np.concatenate(slabs, axis=1))

    in_maps = []
    for c in range(N_CORES):
        in_maps.append({
            "xtb": xtb,
            "wib": np.ascontiguousarray(np.concatenate(
                [pack_wi_v3(wi[j], c) for j in range(N_EXPERTS)], axis=1)),
            "wgb": np.ascontiguousarray(np.concatenate(
                [pack_wi_v3(wg[j], c) for j in range(N_EXPERTS)], axis=1)),
            "wob": np.ascontiguousarray(np.concatenate(
                [pack_wo_v3(wo[j], c) for j in range(N_EXPERTS)], axis=1)),
        })
    return idx_lists, pos, caps, in_maps


def kernel(x, w_gate, wi, wg, wo):
    x = np.asarray(x)
    wi, wg, wo = np.asarray(wi), np.asarray(wg), np.asarray(wo)

    topk_vals, topk_idx = _route(x, w_gate)
    idx_lists, pos, caps, in_maps = make_in_maps_v3(x, wi, wg, wo, topk_idx)

    nc = build_program_v3(caps)
    res = run_bass_kernel_spmd(nc, in_maps, core_ids=list(range(N_CORES)))

    scap = sum(caps)
    capoff = np.cumsum([0] + caps[:-1])
    ysum = np.zeros((C, scap), dtype=np.float32)
    for c in range(N_CORES):
        ysum += res.results[c]["ytp"].astype(np.float32)

    # combine: out[t] = sum_k vals[t,k] * y_{idx[t,k]}[t]
    Y = np.empty((N_EXPERTS, max(caps), C), dtype=np.float32)
    for j in range(N_EXPERTS):
        Y[j, :caps[j]] = ysum[:, capoff[j]:capoff[j] + caps[j]].T
    out = (topk_vals[:, 0:1] * Y[topk_idx[:, 0], pos[:, 0], :]
           + topk_vals[:, 1:2] * Y[topk_idx[:, 1], pos[:, 1], :])
    return out.reshape(B, T, C).astype(np.float32)


# revision 20
# speedup vs baseline: 3.2954x; 1.1658x over previous
"""MoE (top-2 of 8 experts) SwiGLU FFN on 8 Trainium2 NeuronCores.

Strategy — expert-parallel with K-way H-sharding for load balance:
  - Router (x @ w_gate -> softmax -> top-2) computed host-side on jax-CPU
    with the exact ops the reference uses, so expert selection matches the
    reference bit-for-bit ("dispatch tokens by topk_idx").
  - Each expert's H dimension is split over K cores; the SPMD program runs
    NE*K/8 slots.  Slot k covers 8/K experts (grouped by sorted load so the
    slot cap = group max is tight); core c runs expert group-member c//K
    with H-chunk c%K.  K trades PE-cycle balance (larger K -> caps approach
    the per-core arithmetic floor) against duplicated x traffic and
    partial-output volume (K-fold).  K=2 keeps the instruction structure of
    plain expert-parallel while cutting the cap padding from
    8*max_load to 4*(l1+l5) tokens per core.
  - Device computes, per slot, partial y^T = wo_c^T @ (silu(wg_c^T x^T) *
    (wi_c^T x^T)) with bf16 matmuls accumulating in fp32 PSUM.  Tokens stay
    on the PSUM free dimension throughout, so no on-device transposes are
    needed.  Partial y (this core's H-chunk contribution) is written bf16
    via per-(slot, out-chunk) coalesced strip DMAs (many small output DMAs
    saturate the issuing queue and stall PSUM drain); the host sums the K
    partials per expert in fp32 and applies the top-2 combine weights.
  - Perf notes baked in: chip-level power arbitration throttles the PE
    clock when all 8 cores stream matmuls (2.4 -> ~1.9 GHz), so total PE
    cycles and total chip activity (DMA bytes, DVE element work) both
    matter; warm-up matmuls cover the initial DMA wait and pre-warm the
    HAM clock gate; weight layouts are hh-major so the first matmul group
    needs only one small DMA chunk.
"""

import numpy as np
import ml_dtypes

import concourse.bass as bass
import concourse.mybir as mybir
import concourse.tile as tile
from concourse.bass_utils import run_bass_kernel_spmd

N_CORES = 8
N_EXPERTS = 8
TOP_K = 2
B, T, C, H = 4, 2048, 1024, 2048
CC = C // 128            # contraction chunks over C
TOK_TILE = 512           # max tokens per PSUM tile (one fp32 bank)
CO = C // 128            # output row chunks
BF16 = mybir.dt.bfloat16

KSH = 2                  # H-shard factor (1, 2, 4 or 8)


def _split_multi_waits(nc, max_waits=1):
    """This walrus build rejects >1 sync-wait per instruction. Peel extra
    waits onto single-wait EventSemaphore instructions inserted just before,
    on the same engine (identical blocking semantics)."""
    n_split = 0
    for fn in nc.m.functions:
        for bb in fn.blocks:
            out = []
            changed = False
            for inst in bb.instructions:
                si = inst.sync_info
                waits = list(si.on_wait) if si is not None else []
                if len(waits) > max_waits:
                    head, keep = waits[:-max_waits], waits[-max_waits:]
                    for j, w in enumerate(head):
                        out.append(mybir.InstEventSemaphore(
                            name=f"{inst.name}-wspl{j}",
                            engine=inst.engine,
                            sync_info=mybir.SyncInfo(on_wait=[w], on_update=[]),
                        ))
                    inst.sync_info = mybir.SyncInfo(
                        on_wait=keep, on_update=list(si.on_update))
                    changed = True
                    n_split += 1
                out.append(inst)
            if changed:
                bb.instructions = out
    return n_split


def tok_tiling(cap):
    """Token tiles with a small first tile (fast first x DMA), the rest
    split evenly."""
    if cap <= 256:
        return [(0, cap)]
    first = 128
    rest = cap - first
    k = -(-rest // TOK_TILE)
    w = -(-rest // (4 * k)) * 4
    out = [(0, first)]
    t0 = first
    while t0 < cap:
        tw = min(w, cap - t0)
        out.append((t0, tw))
        t0 += tw
    return out


def tiling_plain(cap):
    """Even split into ceil(cap/512) tiles (multiple-of-4 widths): avoids
    tiny tail tiles whose matmul groups are all fixed overhead."""
    k = -(-cap // TOK_TILE)
    w = -(-cap // (4 * k)) * 4
    out = []
    t0 = 0
    while t0 < cap:
        tw = min(w, cap - t0)
        out.append((t0, tw))
        t0 += tw
    return out


def slot_tiling(j, cap):
    return tok_tiling(cap) if j == 0 else tiling_plain(cap)


def _geom(K):
    hsh = H // K             # H columns per core per slot
    hshh = hsh // 128        # hh chunks per slot
    nslot = N_EXPERTS * K // N_CORES
    w1blk = hshh * CC * 128  # stage-1 weight cols per slot
    w2blk = CO * hshh * 128  # stage-2 weight cols per slot
    return hsh, hshh, nslot, w1blk, w2blk


def build_program(caps, K=KSH, reps=1):
    """SPMD program over nslot slots; slot k = one expert of its group
    (which one depends on core // K), H-chunk = core % K.

    DRAM inputs (bf16), slot-concatenated along columns:
      xtb [128, CC*sum(caps)]   slot k at CC*capoff_k per slot_tiling(k,cap)
      wib/wgb [128, nslot*w1blk]  slot k: [p,(hh*CC+cc)*128+f] =
                                  wi[e][cc*128+p, q*hsh+hh*128+f]
      wob [128, nslot*w2blk]      slot k: [p,(co*hshh+hh)*128+f] =
                                  wo[e][q*hsh+hh*128+p, co*128+f]
    Output ytp [C, sum(caps)] bf16: partial y^T (this core's H-chunk).
    reps>1 repeats the whole compute (timing use only).
    """
    hsh, hshh, nslot, w1blk, w2blk = _geom(K)
    assert len(caps) == nslot
    scap = sum(caps)
    capoff = [sum(caps[:k]) for k in range(nslot)]

    nc = bass.Bass()
    xtb = nc.dram_tensor("xtb", [128, CC * scap], BF16, kind="ExternalInput")
    wib = nc.dram_tensor("wib", [128, nslot * w1blk], BF16,
                         kind="ExternalInput")
    wgb = nc.dram_tensor("wgb", [128, nslot * w1blk], BF16,
                         kind="ExternalInput")
    wob = nc.dram_tensor("wob", [128, nslot * w2blk], BF16,
                         kind="ExternalInput")
    ytp = nc.dram_tensor("ytp", [C, scap], BF16, kind="ExternalOutput")
    # tiny output: fetching it blocks on program completion without paying
    # the big ytp transfer through the tunnel (timing use)
    done = nc.dram_tensor("done", [1, 8], BF16, kind="ExternalOutput")

    with tile.TileContext(nc) as tc:
        with tc.tile_pool(name="wu", bufs=1) as wu_pool, \
             tc.tile_pool(name="xb", bufs=1) as xb_pool, \
             tc.tile_pool(name="w1", bufs=3) as w1_pool, \
             tc.tile_pool(name="hT", bufs=1) as h_pool, \
             tc.tile_pool(name="w2", bufs=3) as w2_pool, \
             tc.tile_pool(name="sg", bufs=3) as sg_pool, \
             tc.tile_pool(name="yo", bufs=1) as yo_pool, \
             tc.tile_pool(name="psw", bufs=1, space="PSUM") as psw_pool, \
             tc.tile_pool(name="ps", bufs=2, space="PSUM") as ps_pool, \
             tc.tile_pool(name="ps2", bufs=3, space="PSUM") as ps2_pool:

            # warm-up: keep PE busy during the first DMAs (also warms the
            # HAM clock gate); runs on a zeroed scratch tile into a PSUM
            # bank nothing reads
            wup = wu_pool.tile([128, TOK_TILE], BF16, tag="wup")
            nc.vector.memset(wup[:], 0)
            ps_w = psw_pool.tile([128, TOK_TILE], mybir.dt.float32, tag="pw")
            for _ in range(8):
                nc.tensor.matmul(ps_w[:], wup[:, 0:128], wup[:],
                                 start=True, stop=True)

            mcap = max(caps)
            for _rep in range(reps):
                for j in range(nslot):
                    cap = caps[j]
                    tiles = slot_tiling(j, cap)
                    xoff = CC * capoff[j]

                    # slot 0: per-tile loads so the first matmul group only
                    # waits on a small transfer; others: one big DMA,
                    # double-buffered so the load runs a slot ahead
                    xs = xb_pool.tile([128, CC * mcap], BF16,
                                      tag=f"x{j % 2}", name=f"xs{j}")
                    if j == 0:
                        off = xoff
                        for t0, tw in tiles:
                            nc.gpsimd.dma_start(
                                xs[:, CC * t0:CC * t0 + CC * tw],
                                xtb[:, off:off + CC * tw])
                            off += CC * tw
                    else:
                        nc.gpsimd.dma_start(xs[:, :CC * cap],
                                            xtb[:, xoff:xoff + CC * cap])
                    xts = [xs[:, CC * t0:CC * (t0 + tw)]
                           for t0, tw in tiles]

                    hT = h_pool.tile([128, hshh * mcap], BF16, tag="hT")

                    # ---- stage 1: hT = silu(x@wg) * (x@wi), H-chunk ----
                    # weights stream in per-hh chunks (3-deep ring): the
                    # first matmul group waits only on one 256 KB chunk,
                    # and slot boundaries never stall on a big reload
                    WCH = CC * 128
                    for hh in range(hshh):
                        wib_t = w1_pool.tile([128, WCH], BF16, tag="wib")
                        nc.sync.dma_start(
                            wib_t[:], wib[:, j * w1blk + hh * WCH:
                                          j * w1blk + (hh + 1) * WCH])
                        wgb_t = w1_pool.tile([128, WCH], BF16, tag="wgb")
                        nc.sync.dma_start(
                            wgb_t[:], wgb[:, j * w1blk + hh * WCH:
                                          j * w1blk + (hh + 1) * WCH])
                        for ti, (t0, tw) in enumerate(tiles):
                            ps_u = ps_pool.tile([128, TOK_TILE],
                                                mybir.dt.float32, tag="psu")
                            ps_g = ps_pool.tile([128, TOK_TILE],
                                                mybir.dt.float32, tag="psg")
                            for cc in range(CC):
                                nc.tensor.matmul(
                                    ps_u[:, :tw],
                                    wib_t[:, cc * 128:(cc + 1) * 128],
                                    xts[ti][:, cc * tw:(cc + 1) * tw],
                                    start=(cc == 0), stop=(cc == CC - 1))
                            for cc in range(CC):
                                nc.tensor.matmul(
                                    ps_g[:, :tw],
                                    wgb_t[:, cc * 128:(cc + 1) * 128],
                                    xts[ti][:, cc * tw:(cc + 1) * tw],
                                    start=(cc == 0), stop=(cc == CC - 1))
                            sg = sg_pool.tile([128, TOK_TILE],
                                              mybir.dt.float32, tag="sg")
                            nc.scalar.activation(
                                sg[:, :tw], ps_g[:, :tw],
                                mybir.ActivationFunctionType.Silu)
                            nc.vector.tensor_mul(
                                hT[:, hh * mcap + t0: hh * mcap + t0 + tw],
                                ps_u[:, :tw], sg[:, :tw])

                    # ---- stage 2: partial y^T = wo_chunk^T @ hT ----
                    W2CH = hshh * 128
                    for co in range(CO):
                        c0 = co * 128
                        wob_t = w2_pool.tile([128, W2CH], BF16, tag="wob")
                        nc.sync.dma_start(
                            wob_t[:], wob[:, j * w2blk + co * W2CH:
                                          j * w2blk + (co + 1) * W2CH])
                        yo = yo_pool.tile([128, mcap], BF16,
                                          tag=f"yo{co % 2}", name=f"yo{co}")
                        for t0, tw in tiles:
                            ps_y = ps2_pool.tile([128, TOK_TILE],
                                                 mybir.dt.float32, tag="psy")
                            for hh in range(hshh):
                                nc.tensor.matmul(
                                    ps_y[:, :tw],
                                    wob_t[:, hh * 128:(hh + 1) * 128],
                                    hT[:, hh * mcap + t0:
                                       hh * mcap + t0 + tw],
                                    start=(hh == 0), stop=(hh == hshh - 1))
                            nc.vector.tensor_copy(yo[:, t0:t0 + tw],
                                                  ps_y[:, :tw])
                        nc.scalar.dma_start(
                            ytp[c0:c0 + 128, capoff[j]:capoff[j] + cap],
                            yo[:, :cap])
                        if j == nslot - 1 and co == CO - 1:
                            nc.scalar.dma_start(done[0:1, 0:8], yo[0:1, 0:8])
    _split_multi_waits(nc)
    return nc


def pack_wi(w_e, q, K=KSH):
    """wi/wg [C, H] f32, H-chunk q of K -> [128, w1blk] bf16, hh-major."""
    hsh, hshh, _, w1blk, _ = _geom(K)
    sl = np.asarray(w_e)[:, q * hsh:(q + 1) * hsh]         # [C, hsh]
    a = sl.reshape(CC, 128, hshh, 128)                     # [cc, p, hh, f]
    a = a.transpose(1, 2, 0, 3)                            # [p, hh, cc, f]
    return np.ascontiguousarray(a.reshape(128, w1blk)
                                ).astype(ml_dtypes.bfloat16)


def pack_wo(w_e, q, K=KSH):
    """wo [H, C] f32, H-chunk q of K -> [128, w2blk] bf16, co-major."""
    hsh, hshh, _, _, w2blk = _geom(K)
    sl = np.asarray(w_e)[q * hsh:(q + 1) * hsh, :]         # [hsh, C]
    a = sl.reshape(hshh, 128, CO, 128)                     # [hh, p, co, f]
    a = a.transpose(1, 2, 0, 3)                            # [p, co, hh, f]
    return np.ascontiguousarray(a.reshape(128, w2blk)
                                ).astype(ml_dtypes.bfloat16)


def pack_x(x_disp_T, j, cap):
    """x^T slab [C, cap] f32 -> [128, CC*cap] bf16 per slot_tiling(j, cap)."""
    a = x_disp_T.reshape(CC, 128, cap)
    parts = []
    for t0, tw in slot_tiling(j, cap):
        blk = a[:, :, t0:t0 + tw].transpose(1, 0, 2)
        parts.append(blk.reshape(128, CC * tw))
    return np.ascontiguousarray(np.concatenate(parts, axis=1)
                                ).astype(ml_dtypes.bfloat16)


def _route(x, w_gate):
    """Host-side router. Runs the exact reference ops on jax-CPU so the
    top-2 selection and gate values match the reference bit-for-bit."""
    import jax
    import jax.numpy as jnp
    cpu = jax.devices("cpu")[0]
    with jax.default_device(cpu):
        xj = jnp.asarray(np.asarray(x))
        wj = jnp.asarray(np.asarray(w_gate))
        logits = jnp.einsum("btc,ce->bte", xj, wj)
        gates = jax.nn.softmax(logits, axis=-1)
        topk_vals, topk_idx = jax.lax.top_k(gates, TOP_K)
    return (np.asarray(topk_vals).reshape(-1, TOP_K),
            np.asarray(topk_idx).reshape(-1, TOP_K))


def make_in_maps(x, wi, wg, wo, topk_idx, K=KSH):
    """Dispatch + pack per-core inputs.

    Returns (idx_lists, pos, groups, caps, in_maps) where groups[k] lists
    the 8/K experts of slot k (core c runs groups[k][c // K]).
    """
    hsh, hshh, nslot, w1blk, w2blk = _geom(K)
    N = x.shape[0] * x.shape[1] if np.asarray(x).ndim == 3 else x.shape[0]
    x_flat = np.ascontiguousarray(np.asarray(x).reshape(N, C))
    idx_lists = []
    pos = np.empty((N, TOP_K), dtype=np.int64)
    for e in range(N_EXPERTS):
        sel = (topk_idx == e)
        toks = np.flatnonzero(sel.any(axis=1))
        idx_lists.append(toks)
        pos_of = np.full(N, -1, dtype=np.int64)
        pos_of[toks] = np.arange(len(toks))
        for k in range(TOP_K):
            m = sel[:, k]
            pos[m, k] = pos_of[m]

    # group experts by sorted load so each slot's cap (group max) is tight
    gsz = N_EXPERTS // nslot
    order = sorted(range(N_EXPERTS), key=lambda e: -len(idx_lists[e]))
    groups = [order[k * gsz:(k + 1) * gsz] for k in range(nslot)]
    caps = [max(4, -(-max(len(idx_lists[e]) for e in g) // 4) * 4)
            for g in groups]

    xT = np.ascontiguousarray(x_flat.T)

    def xtb_for(experts):
        slabs = []
        for k, e in enumerate(experts):
            toks = idx_lists[e]
            slab = np.zeros((C, caps[k]), dtype=np.float32)
            slab[:, :len(toks)] = xT[:, toks]
            slabs.append(pack_x(slab, k, caps[k]))
        return np.ascontiguousarray(np.concatenate(slabs, axis=1))

    xtbs = [xtb_for([g[side] for g in groups]) for side in range(gsz)]

    in_maps = []
    for c in range(N_CORES):
        side = c // K
        q = c % K
        experts = [g[side] for g in groups]
        in_maps.append({
            "xtb": xtbs[side],
            "wib": np.ascontiguousarray(np.concatenate(
                [pack_wi(wi[e], q, K) for e in experts], axis=1)),
            "wgb": np.ascontiguousarray(np.concatenate(
                [pack_wi(wg[e], q, K) for e in experts], axis=1)),
            "wob": np.ascontiguousarray(np.concatenate(
                [pack_wo(wo[e], q, K) for e in experts], axis=1)),
        })
    return idx_lists, pos, groups, caps, in_maps


def kernel(x, w_gate, wi, wg, wo):
    x = np.asarray(x)
    wi, wg, wo = np.asarray(wi), np.asarray(wg), np.asarray(wo)
    K = KSH

    topk_vals, topk_idx = _route(x, w_gate)
    idx_lists, pos, groups, caps, in_maps = make_in_maps(
        x, wi, wg, wo, topk_idx, K)

    nc = build_program(caps, K)
    res = run_bass_kernel_spmd(nc, in_maps, core_ids=list(range(N_CORES)))

    capoff = np.cumsum([0] + caps[:-1])
    mcap = max(caps)
    # Y[e]: sum of the K H-chunk partials for expert e
    Y = np.zeros((N_EXPERTS, mcap, C), dtype=np.float32)
    for k, g in enumerate(groups):
        for side, e in enumerate(g):
            acc = np.zeros((C, caps[k]), dtype=np.float32)
            for c in range(side * K, side * K + K):
                acc += res.results[c]["ytp"][:,
                                             capoff[k]:capoff[k] + caps[k]
                                             ].astype(np.float32)
            Y[e, :caps[k]] = acc.T
    out = (topk_vals[:, 0:1] * Y[topk_idx[:, 0], pos[:, 0], :]
           + topk_vals[:, 1:2] * Y[topk_idx[:, 1], pos[:, 1], :])
    return out.reshape(B, T, C).astype(np.float32)
